# revision 25
# baseline (speedup 1.0000x reference)
"""Trainium2 Bass kernel for local-window sparse attention.

Problem: B=32, N=1024 tokens (16x64 grid), C=768, 12 heads x 64 dims,
local 7x11 window additive mask, qkv proj + attention + out proj.

Strategy: data-parallel over batch across 8 NeuronCores (4 batches per
core). Per-core kernel keeps activations feature-major ("transposed")
so no on-chip transposes are needed:
  - host pre-transposes x -> xT [768, 1024] (bf16)
  - qT/kT = W_chunk.T @ xT  (feature-major, heads packed 2-per-128-partitions)
  - v produced token-major with an extra all-ones column per head
    (so the PV matmul also produces the softmax denominator as row 64)
  - scoresT[j] = kT_h.T @ qT_h  (keys on partitions, queries on free dim)
    exp via ScalarE (scale=1/8 folded in), 0/1 band mask multiply on VectorE
  - avT = v_ext.T @ attnT accumulated over key tiles, normalized by the
    denominator row, written feature-major
  - out = avT.T @ W_proj + b_proj  (token-major, straight DMA out)

Only key tiles intersecting the local band are computed (j in [qlo..qhi]
per 512-query chunk), and within each (key-tile, query-chunk) pair the
scores matmul / exp / mask / PV matmul are restricted to the in-band
query column subrange.
"""

import numpy as np
import ml_dtypes

import concourse.bass as bass
import concourse.mybir as mybir
import concourse.tile as tile
from concourse import bacc
from concourse.bass import ds, ts
from concourse.bass_utils import run_bass_kernel_spmd

# ---- problem constants (hardcoded; kernel.py must be self-contained) ----
B, N, C = 32, 1024, 768
HEADS, D = 12, 64
H, W, HK, WK = 16, 64, 7, 11
NCORES = 8
BL = B // NCORES  # batches per core
KT = C // 128     # 6 contraction tiles over embed dim
NT = N // 128     # 8 token tiles
BF16 = mybir.dt.bfloat16
F32 = mybir.dt.float32

ROWS_PER_KTILE = 128 // W  # 2 grid rows per 128-token tile
RH = HK // 2               # 3: half-window in grid rows


def _band_tiles(qc, qchunk=512):
    """Key tiles j intersecting the band for query chunk qc (512 queries)."""
    qr0, qr1 = (qchunk // W) * qc, (qchunk // W) * (qc + 1) - 1  # grid rows
    jlo = max(0, (qr0 - RH) // ROWS_PER_KTILE)
    jhi = min(NT - 1, (qr1 + RH) // ROWS_PER_KTILE)
    return list(range(jlo, jhi + 1))


def _qsub(j, qc, qchunk=512):
    """In-band query column subrange [lo, hi) within chunk qc for key tile j.

    Key tile j covers grid rows [2j, 2j+1]; in-band query grid rows are
    [2j - RH, 2j + 1 + RH] clipped to the chunk. Returns offsets relative
    to chunk start, multiples of W=64.
    """
    rows_per_chunk = qchunk // W
    qr_lo = max(ROWS_PER_KTILE * j - RH, rows_per_chunk * qc)
    qr_hi = min(ROWS_PER_KTILE * j + (ROWS_PER_KTILE - 1) + RH,
                rows_per_chunk * (qc + 1) - 1)
    lo = qr_lo * W - qchunk * qc
    hi = (qr_hi + 1) * W - qchunk * qc
    return lo, hi


def build_kernel(nbatch=BL, subrange=True):
    nc = bacc.Bacc(None, target_bir_lowering=False)
    xT_d = nc.declare_dram_parameter("xT", [nbatch, C, N], BF16, isOutput=False)
    wqkv_d = nc.declare_dram_parameter("wqkv", [C, 3 * C], BF16, isOutput=False)
    wproj_d = nc.declare_dram_parameter("wproj", [C, C], BF16, isOutput=False)
    bproj_d = nc.declare_dram_parameter("bproj", [1, C], BF16, isOutput=False)
    maskT_d = nc.declare_dram_parameter("maskT", [N, N], BF16, isOutput=False)
    out_d = nc.declare_dram_parameter("out", [nbatch, N, C], F32, isOutput=True)

    with tile.TileContext(nc) as tc:
        with (
            tc.tile_pool(name="weights", bufs=1) as wpool,
            tc.tile_pool(name="xt", bufs=2) as xpool,
            tc.tile_pool(name="acts", bufs=2) as qkpool,
            tc.tile_pool(name="acts1", bufs=2) as avpool,
            tc.tile_pool(name="attn", bufs=3) as apool,
            tc.tile_pool(name="outs", bufs=2) as opool,
            tc.tile_pool(name="small", bufs=2) as spool,
            tc.tile_pool(name="gemm_ps", bufs=2, space="PSUM") as gemm_ps,
            tc.tile_pool(name="sc_ps", bufs=2, space="PSUM") as scpool,
            tc.tile_pool(name="pv_ps", bufs=2, space="PSUM") as pvpool,
        ):
            # ---- persistent weights in SBUF (xT(0) is DMA'd first,
            # below, so the first matmuls aren't stuck behind the whole
            # weight burst in the HWDGE FIFO) ----
            wqkv_s = wpool.tile([128, KT, 3 * C], BF16)
            wproj_s = wpool.tile([128, KT, C], BF16)
            maskT_s = wpool.tile([128, NT, N], BF16)
            bproj_s = wpool.tile([1, C], BF16)

            def load_weights():
                for j in range(KT):
                    nc.sync.dma_start(wqkv_s[:, j, :], wqkv_d[ds(128 * j, 128), :])
                nc.sync.dma_start(maskT_s[:], maskT_d[:].rearrange("(j p) n -> p j n", p=128))
                nc.sync.dma_start(wproj_s[:], wproj_d[:].rearrange("(j p) f -> p j f", p=128))
                nc.sync.dma_start(bproj_s[:], bproj_d[:])
            ones_s = wpool.tile([1, 128], BF16)
            nc.vector.memset(ones_s[:], 1.0)
            zero65_s = wpool.tile([1, 65], BF16)
            nc.vector.memset(zero65_s[:], 0.0)

            acts = {}

            def load_x(b):
                xT_s = xpool.tile([128, KT, N], BF16, tag="xT", name=f"xT{b}")
                for j in range(KT):
                    nc.sync.dma_start(xT_s[:, j, :], xT_d[b, ds(128 * j, 128), :])
                acts[b] = {"xT": xT_s}

            def qkv_groups(b):
                xT_s = acts[b]["xT"]
                qT_s = qkpool.tile([128, KT, N], BF16, tag="qT", name=f"qT{b}")
                kT_s = qkpool.tile([128, KT, N], BF16, tag="kT", name=f"kT{b}")
                vext_s = qkpool.tile([128, NT, HEADS, D + 1], BF16, tag="vext",
                                     name=f"vext{b}")
                acts[b].update(qT=qT_s, kT=kT_s, vext=vext_s)
                groups = [lambda: nc.vector.memset(vext_s[:, :, :, D:D + 1], 1.0)]

                def qk_group(ft, qc2):
                    dest = qT_s if ft < KT else kT_s
                    p = ft % KT
                    ps = gemm_ps.tile([128, 512], F32, tag="gemm", name="psqk")
                    for j in range(KT):
                        nc.tensor.matmul(
                            ps[:],
                            wqkv_s[:, j, ds(ft * 128, 128)],
                            xT_s[:, j, ds(qc2 * 512, 512)],
                            start=(j == 0), stop=(j == KT - 1),
                        )
                    nc.vector.tensor_copy(dest[:, p, ds(qc2 * 512, 512)], ps[:])

                def v_group(tt, nck):
                    ps = gemm_ps.tile([128, 512], F32, tag="gemm", name="psv")
                    for j in range(KT):
                        nc.tensor.matmul(
                            ps[:, 0:384],
                            xT_s[:, j, ds(tt * 128, 128)],
                            wqkv_s[:, j, ds(2 * C + nck * 384, 384)],
                            start=(j == 0), stop=(j == KT - 1),
                        )
                    nc.vector.tensor_copy(
                        vext_s[:, tt, ds(6 * nck, 6), 0:D],
                        ps[:, 0:384].rearrange("p (h d) -> p h d", d=D),
                    )

                for ft in range(2 * KT):
                    for qc2 in range(2):
                        groups.append(lambda ft=ft, qc2=qc2: qk_group(ft, qc2))
                for tt in range(NT):
                    for nck in range(2):
                        groups.append(lambda tt=tt, nck=nck: v_group(tt, nck))
                return groups

            def attn_part1(b, hp, qc):
                qT_s, kT_s = acts[b]["qT"], acts[b]["kT"]
                vext_s = acts[b]["vext"]
                js = _band_tiles(qc)
                pv = [pvpool.tile([65, 512], F32, tag="pv", name=f"pv{_h}")
                      for _h in range(2)]
                for half in range(2):
                    nc.tensor.matmul(
                        pv[half][:], zero65_s[:], maskT_s[0:1, 0, 0:512],
                        start=True, stop=False, skip_group_check=True,
                    )
                for ji, j in enumerate(js):
                    lo, hi = _qsub(j, qc) if subrange else (0, 512)
                    w = hi - lo
                    sc = scpool.tile([128, 2, 512], F32, tag="sc")
                    et = apool.tile([128, 2, 512], BF16, tag="et")
                    for half in range(2):
                        nc.tensor.matmul(
                            sc[ds(0, 128), half, ds(0, w)],
                            kT_s[ds(64 * half, 64), hp, ds(128 * j, 128)],
                            qT_s[ds(64 * half, 64), hp, ds(512 * qc + lo, w)],
                            start=True, stop=True,
                        )
                    nc.scalar.activation(
                        et[:, :, ds(0, w)], sc[:, :, ds(0, w)],
                        mybir.ActivationFunctionType.Exp, scale=0.125,
                    )
                    nc.vector.tensor_mul(
                        et[:, :, ds(0, w)],
                        et[:, :, ds(0, w)],
                        maskT_s[:, j, ds(512 * qc + lo, w)]
                        .rearrange("p (a n) -> p a n", a=1)
                        .broadcast_to((128, 2, w)),
                    )
                    for half in range(2):
                        nc.tensor.matmul(
                            pv[half][ds(0, 65), ds(lo, w)],
                            vext_s[:, j, 2 * hp + half, 0:65],
                            et[:, half, ds(0, w)],
                            start=False,
                            stop=(j == js[-1]),
                            skip_group_check=True,
                        )
                return pv

            def attn_part2(b, hp, qc, pv):
                avT_s = acts[b]["avT"]
                avu = apool.tile([128, 512], BF16, tag="avu")
                rb = gemm_ps.tile([128, 512], F32, tag="gemm", name="rb")
                for half in range(2):
                    nc.vector.tensor_copy(avu[ds(64 * half, 64), :],
                                          pv[half][0:64, :])
                    rec = spool.tile([1, 512], F32, tag="rec")
                    nc.vector.reciprocal(rec[:], pv[half][64:65, :])
                    recb = spool.tile([1, 512], BF16, tag="recb")
                    nc.vector.tensor_copy(recb[:], rec[:])
                    nc.tensor.matmul(rb[ds(64 * half, 64), :],
                                     ones_s[:, 0:64], recb[:],
                                     start=True, stop=True)
                nc.vector.tensor_mul(
                    avT_s[:, hp, ds(qc * 512, 512)], avu[:], rb[:],
                )

            def proj_groups(b):
                avT_s = acts[b]["avT"]

                def proj_tile(tt):
                    oat = opool.tile([128, C], F32, tag="oat")
                    for nck in range(2):
                        ps = gemm_ps.tile([128, 512], F32, tag="gemm", name="psp")
                        nc.tensor.matmul(
                            ps[:, 0:384], ones_s[:, 0:128],
                            bproj_s[:, ds(nck * 384, 384)],
                            start=True, stop=False,
                        )
                        for j in range(KT):
                            nc.tensor.matmul(
                                ps[:, 0:384],
                                avT_s[:, j, ds(tt * 128, 128)],
                                wproj_s[:, j, ds(nck * 384, 384)],
                                start=False, stop=(j == KT - 1),
                            )
                        nc.vector.tensor_copy(oat[:, ds(nck * 384, 384)],
                                              ps[:, 0:384])
                    nc.sync.dma_start(out_d[b, ds(tt * 128, 128), :], oat[:])

                return [lambda tt=tt: proj_tile(tt) for tt in range(NT)]

            # software pipeline: interleave QKV(b+1) / proj(b-1) groups
            # between attention(b) iterations (emission order only; all
            # per-op code is identical to the serial version)
            from collections import deque
            import math
            load_x(0)
            load_weights()
            for g in qkv_groups(0):
                g()
            pending = deque()
            for b in range(nbatch):
                acts[b]["avT"] = avpool.tile([128, KT, N], BF16, tag="avT",
                                             name=f"avT{b}")
                if b + 1 < nbatch:
                    load_x(b + 1)
                    pending.extend(qkv_groups(b + 1))
                iters = [(hp, qc) for hp in range(KT) for qc in range(2)]

                def fill(k):
                    for _ in range(min(k, len(pending))):
                        pending.popleft()()

                for i, (hp, qc) in enumerate(iters):
                    quota = min(math.ceil(len(pending) / (len(iters) - i)), 4)
                    pv = attn_part1(b, hp, qc)
                    fill(2)
                    attn_part2(b, hp, qc, pv)
                    fill(quota - 2)
                pending.extend(proj_groups(b))
                if b > 0 and b - 1 in acts:
                    del acts[b - 1]
            while pending:
                pending.popleft()()

    nc.compile()
    return nc


def _local_mask_T():
    """Binary (1=in-window) local mask, transposed: maskT[m, n]."""
    m = np.ones((N, H + HK - 1, W + WK - 1), dtype=np.float32)
    for h in range(H):
        for w in range(W):
            m[h * W + w, h:h + HK, w:w + WK] = 0.0
    mp = m[:, HK // 2:H + HK // 2, WK // 2:W + WK // 2].reshape(N, N)
    binm = (mp < 1.0).astype(np.float32)
    return np.ascontiguousarray(binm.T)


_CACHE = {}


def kernel(x, W_qkv, W_proj, b_proj):
    x = np.asarray(x, dtype=np.float32)
    W_qkv = np.asarray(W_qkv, dtype=np.float32)
    W_proj = np.asarray(W_proj, dtype=np.float32)
    b_proj = np.asarray(b_proj, dtype=np.float32)

    if "nc" not in _CACHE:
        _CACHE["nc"] = build_kernel(BL)
    nc = _CACHE["nc"]

    maskT = _local_mask_T().astype(ml_dtypes.bfloat16)
    wqkv = W_qkv.astype(ml_dtypes.bfloat16)
    wproj = W_proj.astype(ml_dtypes.bfloat16)
    bproj = np.ascontiguousarray(b_proj.reshape(1, C)).astype(ml_dtypes.bfloat16)

    in_maps = []
    for c in range(NCORES):
        xs = x[c * BL:(c + 1) * BL]
        xT = np.ascontiguousarray(xs.transpose(0, 2, 1)).astype(ml_dtypes.bfloat16)
        in_maps.append({"xT": xT, "wqkv": wqkv, "wproj": wproj,
                        "bproj": bproj, "maskT": maskT})

    res = run_bass_kernel_spmd(nc, in_maps, core_ids=list(range(NCORES)))
    _CACHE["results"] = res
    out = np.concatenate([res.results[i]["out"] for i in range(NCORES)], axis=0)
    return out.astype(np.float32)
